# revision 14
# baseline (speedup 1.0000x reference)
"""Trainium2 Bass kernel for nn_DependencyNeuralModel (dependency parser scorer).

v2 design (8 NeuronCores, SPMD):
  Encoder: 2-layer BiLSTM over S=512, replicated on every core, chunk-parallel
    (64 chunks x 2 dirs advance lock-step as 128 rows through the PE).
    K_WARM=16 warmup steps; gate order repacked to [i,f,o,g] so the i/f
    half of the recurrent GEMM can overlap the o/g half's activations.
  Arc scores: score(h,m) depends only on the (h,m) pair (dist is a function
    of m-h), so each core builds the 64-row slice of the full SxS score
    table it owns (h sharded), in a transposed layout where the dist term
    is a contiguous slice of a host-built [H, 1023] offset table and the
    head term is a per-partition activation-fused bias.  The per-arc gather
    is then a single GPSIMD ap_gather of scalar (pair) entries from the
    partition-replicated table; host picks the parity lane and unsorts.
  Sib scores: part-sharded; host sorts each core's 16384 parts by the
    (head,mod,sib) 128-chunk combo (64 combos x 3 static tiles), so each
    128-part tile needs only 3 one-hot gather matmuls instead of 12.
Host does only index/layout preparation and final unshard.
"""
import sys
import types

import numpy as np

sys.path.insert(0, "/opt/trn_rl_repo")

import concourse.bass as bass
import concourse.mybir as mybir
from concourse.tile import TileContext
from concourse.masks import make_identity

S = 512
H = 512
A = 262144
ASIB = 131072
NB = 17
L = 8
K_WARM = 16
NSTEP = K_WARM + L  # 24
NC = 8
F32 = mybir.dt.float32
BF16 = mybir.dt.bfloat16
U16 = mybir.dt.uint16
BINS = np.array(list(range(10)) + list(range(10, 40, 5)) + [40], dtype=np.int64)

GPERM = np.r_[0:1024, 1536:2048, 1024:1536]  # gate reorder i,f,g,o -> i,f,o,g


def _install_ntff_hook():
    if "antenv.axon_hooks" in sys.modules:
        return
    mod = types.ModuleType("antenv.axon_hooks")
    state = {"hook": None, "tried": False}

    def set_axon_ntff_profile_hook(hook):
        state["hook"] = hook

    def get_axon_ntff_profile_hook():
        if state["hook"] is None and not state["tried"]:
            state["tried"] = True
            try:
                from trn_agent_boot.trn_boot import _ntff_profile_via_ctypes

                state["hook"] = _ntff_profile_via_ctypes("/opt/axon/libaxon_pjrt.so")
            except Exception:
                state["hook"] = None
        return state["hook"]

    mod.set_axon_ntff_profile_hook = set_axon_ntff_profile_hook
    mod.get_axon_ntff_profile_hook = get_axon_ntff_profile_hook
    import antenv

    antenv.axon_hooks = mod
    sys.modules["antenv.axon_hooks"] = mod


def _legalize_waits(nc):
    """This walrus accepts at most one semaphore wait per instruction;
    split extra waits onto same-engine NOPs placed just before."""
    ctr = [0]
    for f in nc.m.functions:
        for blk in f.blocks:
            out = []
            dirty = False
            for ins in blk.instructions:
                si = ins.sync_info
                if si is not None and si.on_wait and len(si.on_wait) > 1:
                    waits = list(si.on_wait)
                    for w in waits[:-1]:
                        ctr[0] += 1
                        nop = mybir.InstNoOp(name=f"waitfix-{ctr[0]}")
                        nop.engine = ins.engine
                        nop.sync_info = mybir.SyncInfo(on_wait=[w], on_update=[])
                        out.append(nop)
                    ins.sync_info = mybir.SyncInfo(
                        on_wait=[waits[-1]],
                        on_update=list(si.on_update) if si.on_update else [],
                    )
                    dirty = True
                out.append(ins)
            if dirty:
                blk.instructions = out
    return nc


def _lstm_layer(nc, tc, ident, mask_sb, whhT_dram, wx_dram, f_dram, b_dram):
    """One BiLSTM layer, chunk-parallel.  B=128 rows: partitions 0:64 are
    dir0 chunks, 64:128 dir1 chunks.  Gate columns are [i,f,o,g]; the
    i/f half of the recurrent GEMM is emitted first so its activations
    overlap the o/g half's matmuls."""
    import contextlib

    with contextlib.ExitStack() as ctx:
        sb = ctx.enter_context(tc.tile_pool(name="lstm_sb", bufs=3))
        cold = ctx.enter_context(tc.tile_pool(name="lstm_cold", bufs=1))
        st = ctx.enter_context(tc.tile_pool(name="lstm_state", bufs=1))
        ps1 = ctx.enter_context(tc.tile_pool(name="lstm_ps1", bufs=1, space="PSUM"))
        ps2 = ctx.enter_context(tc.tile_pool(name="lstm_ps2", bufs=1, space="PSUM"))
        pst = ctx.enter_context(tc.tile_pool(name="lstm_pst", bufs=4, space="PSUM"))

        whh_sb = st.tile([128, 4, 2, 2048], BF16)
        nc.sync.dma_start(
            whh_sb.rearrange("p a b c -> p (a b c)"),
            whhT_dram.rearrange("p a b c -> p (a b c)"),
        )
        h_t = st.tile([128, 4, 128], BF16)  # h transposed: [k-part, kc, b]
        c_st = st.tile([128, 512], F32)     # [b, k]
        nc.vector.memset(h_t.rearrange("p a b -> p (a b)"), 0.0)
        nc.vector.memset(c_st[:], 0.0)

        for s in range(NSTEP):
            wx = sb.tile([128, 2048], BF16, tag="wx")
            for d in range(2):
                nc.sync.dma_start(
                    wx[d * 64:(d + 1) * 64, :], wx_dram[d, s:s + 505:8, :]
                )
            # gates psum = I.T @ wx (identity injects wx) + h @ whh.
            # d0 writes psum rows 0:64 (array cols 0-63), d1 rows 64:128
            # (cols 64-127); adjacent d0/d1 matmuls run concurrently as
            # (128,64) column tiles, halving PE wall time.
            g01 = ps1.tile([128, 1024], F32, tag="g01")
            g23 = ps2.tile([128, 1024], F32, tag="g23")
            for half, gps in ((0, g01), (1, g23)):
                for ng in range(2):
                    col = (half * 2 + ng) * 512
                    for d in range(2):
                        bs = slice(d * 64, (d + 1) * 64)
                        nc.tensor.matmul(
                            gps[bs, ng * 512:(ng + 1) * 512],
                            lhsT=ident[:, bs],
                            rhs=wx[:, col:col + 512],
                            start=True, stop=False,
                        )
                    for kc in range(4):
                        for d in range(2):
                            bs = slice(d * 64, (d + 1) * 64)
                            nc.tensor.matmul(
                                gps[bs, ng * 512:(ng + 1) * 512],
                                lhsT=h_t[:, kc, bs],
                                rhs=whh_sb[:, kc, d, col:col + 512],
                                start=False,
                                stop=(kc == 3),
                            )
            sig_if = cold.tile([128, 1024], BF16, tag="sif")
            nc.scalar.activation(sig_if[:], g01[:],
                                 mybir.ActivationFunctionType.Sigmoid)
            tanh_g = cold.tile([128, 512], BF16, tag="tg")
            nc.scalar.activation(tanh_g[:], g23[:, 512:1024],
                                 mybir.ActivationFunctionType.Tanh)
            sig_o = cold.tile([128, 512], BF16, tag="so")
            nc.scalar.activation(sig_o[:], g23[:, 0:512],
                                 mybir.ActivationFunctionType.Sigmoid)
            t1 = cold.tile([128, 512], F32, tag="t1")
            nc.vector.tensor_mul(t1[:], sig_if[:, 512:1024], c_st[:])
            t2 = cold.tile([128, 512], BF16, tag="t2")
            nc.vector.tensor_mul(t2[:], sig_if[:, 0:512], tanh_g[:])
            nc.vector.tensor_add(c_st[:], t1[:], t2[:])
            tch = cold.tile([128, 512], BF16, tag="tch")
            nc.scalar.activation(tch[:], c_st[:], mybir.ActivationFunctionType.Tanh)
            h_new = cold.tile([128, 512], BF16, tag="h")
            nc.vector.tensor_mul(h_new[:], sig_o[:], tch[:])
            if s in (7, 15) and s < K_WARM:
                mi = {7: 0, 15: 1}[s]
                nc.vector.tensor_scalar_mul(h_new[:], h_new[:], mask_sb[:, mi:mi + 1])
                nc.vector.tensor_scalar_mul(c_st[:], c_st[:], mask_sb[:, mi:mi + 1])
            for kc in range(4):
                tp = pst.tile([128, 128], BF16, tag="tr_ps")
                nc.tensor.transpose(tp[:], h_new[:, kc * 128:(kc + 1) * 128], ident[:])
                nc.vector.tensor_copy(h_t[:, kc, :], tp[:])
            if s >= K_WARM:
                o = s - K_WARM
                nc.sync.dma_start(f_dram[o:505 + o:8, :], h_new[0:64, :])
                nc.sync.dma_start(b_dram[o:505 + o:8, :], h_new[64:128, :])


def _transpose_pair(nc, tc, ident, rev, f_dram, b_dram, dstT, dstTrev, one_row):
    """Build [feat, pos] lhsT chunks (and optionally pos-reversed copy) from
    the per-direction output buffers.  dstT/dstTrev: [128, 9, 512] tiles;
    chunk 8 row 0 is set to ones (bias); rest of chunk 8 zero."""
    import contextlib

    with contextlib.ExitStack() as ctx:
        sb = ctx.enter_context(tc.tile_pool(name="tp_sb", bufs=3))
        ps = ctx.enter_context(tc.tile_pool(name="tp_ps", bufs=2, space="PSUM"))
        for dst in (dstT, dstTrev):
            if dst is None:
                continue
            nc.vector.memset(dst[:, 8, :], 0.0)
            nc.vector.tensor_copy(dst[0:1, 8, :], one_row[:])
        for j in range(4):
            for pc in range(4):
                fsrc = sb.tile([128, 128], BF16, tag="fsrc")
                nc.sync.dma_start(fsrc[:], f_dram[pc * 128:(pc + 1) * 128,
                                                  j * 128:(j + 1) * 128])
                tp = ps.tile([128, 128], BF16, tag="tp")
                nc.tensor.transpose(tp[:], fsrc[:], ident[:])
                nc.vector.tensor_copy(dstT[:, j, pc * 128:(pc + 1) * 128], tp[:])
                if dstTrev is not None:
                    tpr = ps.tile([128, 128], BF16, tag="tpr")
                    nc.tensor.transpose(tpr[:], fsrc[:], rev[:])
                    nc.vector.tensor_copy(
                        dstTrev[:, j, (3 - pc) * 128:(4 - pc) * 128], tpr[:])
                bsrc = sb.tile([128, 128], BF16, tag="bsrc")
                nc.sync.dma_start(bsrc[:], b_dram[pc * 128:(pc + 1) * 128,
                                                  j * 128:(j + 1) * 128])
                # b rows are scan order q; position = 511-q: reverse via rev
                tpb = ps.tile([128, 128], BF16, tag="tpb")
                nc.tensor.transpose(tpb[:], bsrc[:], rev[:])
                nc.vector.tensor_copy(
                    dstT[:, 4 + j, (3 - pc) * 128:(4 - pc) * 128], tpb[:])
                if dstTrev is not None:
                    tpb2 = ps.tile([128, 128], BF16, tag="tpb2")
                    nc.tensor.transpose(tpb2[:], bsrc[:], ident[:])
                    nc.vector.tensor_copy(
                        dstTrev[:, 4 + j, pc * 128:(pc + 1) * 128], tpb2[:])


def _input_gemm(nc, tc, lhsT_tiles, wihT_dram, wx_dram, nk, klast):
    """WX[d] = lhsT_d.T @ wihT[d] -> wx_dram[d, K_WARM:K_WARM+512, :].
    lhsT_tiles: per-dir tile [128, nk, 512] in SBUF ([feat-part, chunk, pos]).
    nk chunks; last chunk has klast valid rows."""
    import contextlib

    with contextlib.ExitStack() as ctx:
        sb = ctx.enter_context(tc.tile_pool(name="ig_sb", bufs=6))
        ps = ctx.enter_context(tc.tile_pool(name="ig_ps", bufs=2, space="PSUM"))
        for d in range(2):
            lhsT = lhsT_tiles[d]
            for ngc in range(4):
                acc4 = ps.tile([128, 4, 512], F32, tag="acc4")
                for kc in range(nk):
                    kk = 128 if kc < nk - 1 else klast
                    rhs = sb.tile([128, 512], wihT_dram.dtype, tag="rhs")
                    nc.sync.dma_start(
                        rhs[:kk, :],
                        wihT_dram[kc * 128:kc * 128 + kk, d,
                                  ngc * 512:(ngc + 1) * 512],
                    )
                    for mc in range(4):
                        nc.tensor.matmul(
                            acc4[:, mc, :],
                            lhsT=lhsT[:kk, kc, mc * 128:(mc + 1) * 128],
                            rhs=rhs[:kk, :],
                            start=(kc == 0),
                            stop=(kc == nk - 1),
                        )
                osb = sb.tile([128, 4, 512], BF16, tag="osb")
                nc.vector.tensor_copy(
                    osb.rearrange("p a b -> p (a b)"),
                    acc4.rearrange("p a b -> p (a b)"))
                for mc in range(4):
                    nc.sync.dma_start(
                        wx_dram[d, K_WARM + mc * 128:K_WARM + (mc + 1) * 128,
                                ngc * 512:(ngc + 1) * 512],
                        osb[:, mc, :],
                    )


def _build(nc, sib_combos, arc_buckets):
    dt = F32
    n_sib_tile = len(sib_combos)       # even
    n_arc_tile = len(arc_buckets)      # even
    n_tile = n_sib_tile + n_arc_tile
    embT_f = nc.dram_tensor("embT_f", [128, 3, 512], BF16, kind="ExternalInput")
    embT_b = nc.dram_tensor("embT_b", [128, 3, 512], BF16, kind="ExternalInput")
    wih0T = nc.dram_tensor("wih0T", [384, 2, 2048], BF16, kind="ExternalInput")
    whh0T = nc.dram_tensor("whh0T", [128, 4, 2, 2048], BF16, kind="ExternalInput")
    wih1T = nc.dram_tensor("wih1T", [1152, 2, 2048], BF16, kind="ExternalInput")
    whh1T = nc.dram_tensor("whh1T", [128, 4, 2, 2048], BF16, kind="ExternalInput")
    projT = nc.dram_tensor("projT", [1152, 2560], BF16, kind="ExternalInput")
    dwin_in = nc.dram_tensor("dwin_in", [128, 4, 576], BF16, kind="ExternalInput")
    hsel_in = nc.dram_tensor("hsel_in", [128, 4, 64], BF16, kind="ExternalInput")
    wrep_in = nc.dram_tensor("wrep_in", [128, 512], BF16, kind="ExternalInput")
    wrepT_in = nc.dram_tensor("wrepT_in", [128, 4, 128], BF16, kind="ExternalInput")
    sib_oh_in = nc.dram_tensor("sib_oh_in", [n_sib_tile // 2, 128, 768], BF16,
                               kind="ExternalInput")
    arc_oh_in = nc.dram_tensor("arc_oh_in", [n_arc_tile // 2, 64, 256], BF16,
                               kind="ExternalInput")
    arcm_in = nc.dram_tensor("arcm_in", [128, n_arc_tile], dt,
                             kind="ExternalInput")
    iotar_in = nc.dram_tensor("iotar_in", [128, 128], dt, kind="ExternalInput")
    mask_in = nc.dram_tensor("mask_in", [128, 2], dt, kind="ExternalInput")
    rev_in = nc.dram_tensor("rev_in", [128, 128], BF16, kind="ExternalInput")
    scores_out = nc.dram_tensor("scores_out", [128, n_tile], dt,
                                kind="ExternalOutput")

    wx0 = nc.dram_tensor("wx0", [2, 544, 2048], BF16)
    wx1 = nc.dram_tensor("wx1", [2, 544, 2048], BF16)
    tdram = nc.dram_tensor("tdram", [64, 512], BF16)
    f0d = nc.dram_tensor("f0d", [512, 512], BF16)
    b0d = nc.dram_tensor("b0d", [512, 512], BF16)
    f1d = nc.dram_tensor("f1d", [512, 512], BF16)
    b1d = nc.dram_tensor("b1d", [512, 512], BF16)

    import contextlib

    with TileContext(nc) as tc:
        with contextlib.ExitStack() as ctx:
            const = ctx.enter_context(tc.tile_pool(name="const", bufs=1))
            big = ctx.enter_context(tc.tile_pool(name="big", bufs=1))

            ident = const.tile([128, 128], BF16)
            make_identity(nc, ident[:])
            rev = const.tile([128, 128], BF16)
            nc.sync.dma_start(rev[:], rev_in[:])
            mask_sb = const.tile([128, 2], dt)
            nc.sync.dma_start(mask_sb[:], mask_in[:])
            one_row = const.tile([1, 512], BF16)
            nc.vector.memset(one_row[:], 1.0)
            wrep_sb = const.tile([128, 512], BF16)
            nc.sync.dma_start(wrep_sb[:], wrep_in[:])
            wrepT_sb = const.tile([128, 4, 128], BF16)
            nc.sync.dma_start(wrepT_sb.rearrange("p a b -> p (a b)"),
                              wrepT_in.rearrange("p a b -> p (a b)"))
            dwin_sb = const.tile([128, 4, 576], BF16)
            nc.sync.dma_start(dwin_sb.rearrange("p a b -> p (a b)"),
                              dwin_in.rearrange("p a b -> p (a b)"))
            hsel_sb = const.tile([128, 4, 64], BF16)
            nc.sync.dma_start(hsel_sb.rearrange("p a b -> p (a b)"),
                              hsel_in.rearrange("p a b -> p (a b)"))
            iota_row = const.tile([128, 128], dt)
            nc.sync.dma_start(iota_row[:], iotar_in[:])
            arcm_sb = const.tile([128, n_arc_tile], dt)
            nc.sync.dma_start(arcm_sb[:], arcm_in[:])

            # zero-pad warmup rows of WX buffers
            with tc.tile_pool(name="zp", bufs=1) as zp:
                zrow = zp.tile([64, 2048], BF16)
                nc.vector.memset(zrow[:], 0.0)
                for wxd in (wx0, wx1):
                    for d in range(2):
                        nc.sync.dma_start(wxd[d, 0:K_WARM, :], zrow[0:K_WARM, :])
                        nc.sync.dma_start(wxd[d, K_WARM + 512:544, :],
                                          zrow[0:32 - K_WARM, :])

            # ---- WX0 ----
            with tc.tile_pool(name="emb_sb", bufs=1) as emb_pool:
                ef = emb_pool.tile([128, 3, 512], BF16)
                nc.sync.dma_start(ef.rearrange("p a b -> p (a b)"),
                                  embT_f.rearrange("p a b -> p (a b)"))
                eb = emb_pool.tile([128, 3, 512], BF16)
                nc.sync.dma_start(eb.rearrange("p a b -> p (a b)"),
                                  embT_b.rearrange("p a b -> p (a b)"))
                _input_gemm(nc, tc, [ef, eb], wih0T, wx0, 3, 128)

            # ---- layer 0 ----
            _lstm_layer(nc, tc, ident, mask_sb, whh0T, wx0, f0d, b0d)

            # ---- x1T / x1Trev ----
            x1T = big.tile([128, 9, 512], BF16, tag="x1T")
            x1Trev = big.tile([128, 9, 512], BF16, tag="x1Trev")
            _transpose_pair(nc, tc, ident, rev, f0d, b0d, x1T, x1Trev, one_row)

            # ---- WX1 ----
            _input_gemm(nc, tc, [x1T, x1Trev], wih1T, wx1, 9, 1)

            # ---- layer 1 ----
            _lstm_layer(nc, tc, ident, mask_sb, whh1T, wx1, f1d, b1d)

            # ---- statesT ----
            stT = big.tile([128, 9, 512], BF16, tag="x1T")  # reuse x1T slot
            _transpose_pair(nc, tc, ident, rev, f1d, b1d, stT, None, one_row)

            # ---- pos-major projection tables (head + 3 sib; skip mod) ----
            tables_sb = big.tile([128, 4, 2560], BF16, tag="tables")
            with contextlib.ExitStack() as c2:
                sb2 = c2.enter_context(tc.tile_pool(name="tb_sb", bufs=6))
                with tc.tile_pool(name="tb_ps4", bufs=2, space="PSUM") as ps4:
                    for ngc in (0, 2, 3, 4):
                        acc4 = ps4.tile([128, 4, 512], dt, tag="acc4")
                        for kc in range(9):
                            kk = 128 if kc < 8 else 1
                            rhs = sb2.tile([128, 512], BF16, tag="rhs")
                            nc.sync.dma_start(
                                rhs[:kk, :],
                                projT[kc * 128:kc * 128 + kk,
                                      ngc * 512:(ngc + 1) * 512],
                            )
                            for mc in range(4):
                                nc.tensor.matmul(
                                    acc4[:, mc, :],
                                    lhsT=stT[:kk, kc, mc * 128:(mc + 1) * 128],
                                    rhs=rhs[:kk, :],
                                    start=(kc == 0),
                                    stop=(kc == 8),
                                )
                        for mc in range(4):
                            nc.vector.tensor_copy(
                                tables_sb[:, mc, ngc * 512:(ngc + 1) * 512],
                                acc4[:, mc, :])
                ps2 = c2.enter_context(tc.tile_pool(name="tb_ps", bufs=2,
                                                    space="PSUM"))

                # ---- transposed mod table M_T[j, m] ----
                mT = big.tile([128, 4, 512], BF16, tag="mT")
                for jc in range(4):
                    acc = ps2.tile([128, 512], dt, tag="acc")
                    for kc in range(8):
                        lh = sb2.tile([128, 128], BF16, tag="lh")
                        nc.sync.dma_start(
                            lh[:],
                            projT[kc * 128:(kc + 1) * 128,
                                  512 + jc * 128:512 + (jc + 1) * 128],
                        )
                        nc.tensor.matmul(
                            acc[:], lhsT=lh[:], rhs=stT[:, kc, :],
                            start=(kc == 0), stop=(kc == 7),
                        )
                    nc.vector.tensor_copy(mT[:, jc, :], acc[:])

                # ---- H window: hwin[j, hl] = heads[64c+hl, j] ----
                hwin = big.tile([128, 4, 64], dt, tag="hwin")
                for jc in range(4):
                    acc = ps2.tile([128, 64], dt, tag="acch")
                    for kc in range(4):
                        nc.tensor.matmul(
                            acc[:],
                            lhsT=tables_sb[:, kc, jc * 128:(jc + 1) * 128],
                            rhs=hsel_sb[:, kc, :],
                            start=(kc == 0), stop=(kc == 3),
                        )
                    nc.vector.tensor_copy(hwin[:, jc, :], acc[:])

            # ---- scoring ----
            scores_sb = big.tile([128, n_tile], dt, tag="scores")
            with contextlib.ExitStack() as c3:
                sb3 = c3.enter_context(tc.tile_pool(name="sc_sb", bufs=4))
                ps_sacc = c3.enter_context(tc.tile_pool(name="ps_sacc", bufs=2,
                                                        space="PSUM"))

                def sib_pair(p):
                    # sib tiles 2p, 2p+1 with host-uploaded one-hots
                    oh = sb3.tile([128, 2, 3, 128], BF16, tag="oh")
                    nc.sync.dma_start(
                        oh.rearrange("p a b c -> p (a b c)"),
                        sib_oh_in[p, :, :],
                    )
                    sacc2 = ps_sacc.tile([128, 2, 512], dt, tag="sacc2")
                    for h2 in range(2):
                        combo = sib_combos[2 * p + h2]
                        chunks = (combo // 16, (combo // 4) % 4, combo % 4)
                        for g in range(3):
                            nc.tensor.matmul(
                                sacc2[:, h2, :], lhsT=oh[:, h2, g, :],
                                rhs=tables_sb[:, chunks[g],
                                              1024 + g * 512:
                                              1024 + (g + 1) * 512],
                                start=(g == 0), stop=(g == 2),
                            )
                    th2 = sb3.tile([128, 2, 512], BF16, tag="th2")
                    nc.scalar.activation(
                        th2.rearrange("p a b -> p (a b)"),
                        sacc2.rearrange("p a b -> p (a b)"),
                        mybir.ActivationFunctionType.Tanh)
                    for h2 in range(2):
                        junk = sb3.tile([128, 512], BF16, tag="junk")
                        nc.vector.scalar_tensor_tensor(
                            junk[:], th2[:, h2, :], 1.0, wrep_sb[:],
                            op0=mybir.AluOpType.mult,
                            op1=mybir.AluOpType.mult,
                            accum_out=scores_sb[:, 2 * p + h2:2 * p + h2 + 1],
                        )

                N_PAIR_A = 16
                # phase A: table rows interleaved with first sib pairs
                with tc.tile_pool(name="ps_tblw", bufs=2, space="PSUM") as ps_w:
                    for hl in range(64):
                        tmp = sb3.tile([128, 4, 512], BF16, tag="tmp")
                        for jc in range(4):
                            nc.vector.scalar_tensor_tensor(
                                tmp[:, jc, :],
                                dwin_sb[:, jc, 63 - hl:63 - hl + 512],
                                hwin[:, jc, hl:hl + 1],
                                mT[:, jc, :],
                                op0=mybir.AluOpType.add,
                                op1=mybir.AluOpType.add)
                        tht = sb3.tile([128, 4, 512], BF16, tag="tht")
                        nc.scalar.activation(
                            tht.rearrange("p a b -> p (a b)"),
                            tmp.rearrange("p a b -> p (a b)"),
                            mybir.ActivationFunctionType.Tanh)
                        wps = ps_w.tile([128, 512], dt, tag="wps")
                        for jc in range(4):
                            nc.tensor.matmul(
                                wps[:], lhsT=wrepT_sb[:, jc, :],
                                rhs=tht[:, jc, :],
                                start=(jc == 0), stop=(jc == 3),
                            )
                        trow = sb3.tile([128, 512], BF16, tag="trow")
                        nc.vector.tensor_copy(trow[:], wps[:])
                        nc.sync.dma_start(tdram[hl:hl + 1, :], trow[0:1, :])
                        if hl < N_PAIR_A:
                            sib_pair(hl)

                table_hm = big.tile([64, 512], BF16, tag="table_hm")
                nc.sync.dma_start(table_hm[:], tdram[:])

                # phase B: remaining sib pairs + arc gather tile pairs
                with tc.tile_pool(name="ps_arc", bufs=2, space="PSUM") as ps_a:

                    def arc_pair(pa):
                        # arc tiles u=2pa, 2pa+1 with host-uploaded h one-hots
                        u = 2 * pa
                        ohh = sb3.tile([64, 2, 128], BF16, tag="ohh")
                        nc.sync.dma_start(
                            ohh.rearrange("p a b -> p (a b)"),
                            arc_oh_in[pa, :, :],
                        )
                        comb = ps_a.tile([128, 2, 128], dt, tag="comb")
                        for h2 in range(2):
                            bucket = arc_buckets[u + h2]
                            nc.tensor.matmul(
                                comb[:, h2, :], lhsT=ohh[:, h2, :],
                                rhs=table_hm[0:64,
                                             bucket * 128:(bucket + 1) * 128],
                                start=True, stop=True,
                            )
                        for h2 in range(2):
                            junk2 = sb3.tile([128, 128], BF16, tag="junk2")
                            nc.vector.scalar_tensor_tensor(
                                junk2[:], iota_row[:],
                                arcm_sb[:, u + h2:u + h2 + 1],
                                comb[:, h2, :],
                                op0=mybir.AluOpType.is_equal,
                                op1=mybir.AluOpType.mult,
                                accum_out=scores_sb[:, n_sib_tile + u + h2:
                                                    n_sib_tile + u + h2 + 1],
                            )

                    n_sib_pair = n_sib_tile // 2
                    n_arc_pair = n_arc_tile // 2
                    nxt = 0
                    nb = n_sib_pair - N_PAIR_A
                    for k in range(nb):
                        sib_pair(N_PAIR_A + k)
                        na = (n_arc_pair * (k + 1)) // nb - (n_arc_pair * k) // nb
                        for _ in range(na):
                            if nxt < n_arc_pair:
                                arc_pair(nxt)
                                nxt += 1
                    while nxt < n_arc_pair:
                        arc_pair(nxt)
                        nxt += 1

                nc.sync.dma_start(scores_out[:], scores_sb[:])
    return nc


_CACHE = {}


def _get_program(sib_combos, arc_buckets):
    key = (tuple(sib_combos), tuple(arc_buckets))
    if _CACHE.get("key") != key:
        nc = bass.Bass()
        _build(nc, sib_combos, arc_buckets)
        _legalize_waits(nc)
        _CACHE["nc"] = nc
        _CACHE["key"] = key
    return _CACHE["nc"]


def _host_prepare(inputs):
    import jax.numpy as jnp
    import ml_dtypes
    _BF = ml_dtypes.bfloat16

    def bf(x):
        return np.asarray(jnp.asarray(np.asarray(x, np.float32), jnp.bfloat16))

    f32 = np.float32
    words = np.asarray(inputs["words"]).astype(np.int64)
    tags = np.asarray(inputs["tags"]).astype(np.int64)
    word_emb = np.asarray(inputs["word_emb"], f32)
    tag_emb = np.asarray(inputs["tag_emb"], f32)
    emb = np.concatenate([word_emb[words], tag_emb[tags]], axis=-1)  # [512, 364]
    emb_aug = np.concatenate([emb, np.ones((S, 1), f32)], axis=1)    # [512, 365]

    def packT(x, rows):  # -> [rows(pad), ...] = x.T zero-padded
        out = np.zeros((rows, x.shape[0]), f32)
        out[: x.shape[1]] = x.T
        return out

    embT_f = bf(packT(emb_aug, 384).reshape(3, 128, 512).transpose(1, 0, 2))
    embT_b = bf(packT(emb_aug[::-1], 384).reshape(3, 128, 512).transpose(1, 0, 2))

    def wih_pack(Wih, bih, bhh, kdim, rows):
        out = np.zeros((rows, 2, 4 * H), f32)
        for d in range(2):
            out[:kdim, d] = np.asarray(Wih[d], f32).T[:, GPERM]
            out[kdim, d] = (np.asarray(bih[d], f32) + np.asarray(bhh[d], f32))[GPERM]
        return out

    wih0T = bf(wih_pack(inputs["Wih0"], inputs["bih0"], inputs["bhh0"], 364, 384))
    wih1T = bf(wih_pack(inputs["Wih1"], inputs["bih1"], inputs["bhh1"], 1024, 1152))

    def whh_pack(Whh):
        out = np.zeros((128, 4, 2, 4 * H), f32)
        for d in range(2):
            wt = np.asarray(Whh[d], f32).T[:, GPERM]  # [512 k, 2048 g]
            out[:, :, d, :] = wt.reshape(4, 128, 4 * H).transpose(1, 0, 2)
        return out

    whh0T = bf(whh_pack(inputs["Whh0"]))
    whh1T = bf(whh_pack(inputs["Whh1"]))

    projs = [inputs["head_W"], inputs["mod_W"], inputs["sib_head_W"],
             inputs["sib_mod_W"], inputs["sib_sib_W"]]
    projT = np.zeros((1152, 5 * H), f32)
    for i, W in enumerate(projs):
        projT[:1024, i * H:(i + 1) * H] = np.asarray(W, f32).T
    projT = bf(projT)

    w = np.asarray(inputs["arc_w"], f32).reshape(512)
    wrep = bf(np.broadcast_to(w, (128, 512)))
    wrepT = bf(w.reshape(4, 128).T.reshape(128, 4, 1).repeat(128, axis=2))

    # Dfull[off] = D[distidx(off - 511)], off in [0, 1022]
    D = (np.asarray(inputs["dist_emb"], f32) @ np.asarray(inputs["dist_W"], f32).T
         + np.asarray(inputs["dist_b"], f32))
    offs = np.arange(-511, 512)
    bi = np.searchsorted(BINS, np.abs(offs), side="right") - 1
    Dfull = D[np.where(offs > 0, bi, bi + NB)]          # [1023, H]
    DfullT = Dfull.T                                     # [H, 1023]

    iotar = np.tile(np.arange(128, dtype=f32), (128, 1))
    mask = np.zeros((128, 2), f32)
    for mi, s in enumerate((7, 15)):
        c = np.arange(64)
        v = ((8 * c + s) > (K_WARM - 1)).astype(f32)
        mask[0:64, mi] = v
        mask[64:128, mi] = v
    revm = np.zeros((128, 128), f32)
    revm[np.arange(128), 127 - np.arange(128)] = 1.0
    revm = bf(revm)

    base = {
        "embT_f": embT_f, "embT_b": embT_b,
        "wih0T": wih0T, "whh0T": whh0T, "wih1T": wih1T, "whh1T": whh1T,
        "projT": projT, "wrep_in": wrep, "wrepT_in": wrepT,
        "iotar_in": iotar, "mask_in": mask, "rev_in": revm,
    }

    ah = np.asarray(inputs["arc_head"]).astype(np.int64)
    am = np.asarray(inputs["arc_mod"]).astype(np.int64)
    sh_i = np.asarray(inputs["sib_head"]).astype(np.int64)
    sm_i = np.asarray(inputs["sib_mod"]).astype(np.int64)
    ss_i = np.asarray(inputs["sib_sib"]).astype(np.int64)

    # ---- global tile layouts (uniform across cores; program depends on them)
    # sibs are sharded BY COMBO: combo c's parts split evenly over cores, each
    # core gets ceil(cnt_g[c]/(128*NC)) tiles for combo c.
    combo_g = (sh_i // 128) * 16 + (sm_i // 128) * 4 + (ss_i // 128)
    cnt_g = np.bincount(combo_g, minlength=64)
    sib_tpc = -(-cnt_g // (128 * NC))              # tiles per combo per core
    sib_combos = [c for c in range(64) for _ in range(sib_tpc[c])]
    if len(sib_combos) % 2:
        sib_combos.append(int(np.argmax(sib_tpc == 0)) if (sib_tpc == 0).any()
                          else 0)
        sib_pad = 1
    else:
        sib_pad = 0
    n_sib_tile = len(sib_combos)
    sib_tile_off = np.zeros(65, np.int64)          # first tile of each combo
    np.cumsum(sib_tpc, out=sib_tile_off[1:65])
    sib_ids_by_combo = [np.nonzero(combo_g == c)[0] for c in range(64)]

    # arcs stay h-sharded (core owns a 64-row table slice); bucket tile counts
    # take the max over cores so the layout is core-uniform.
    core_of = ah // 64
    arc_ids_core = [np.nonzero(core_of == core)[0] for core in range(NC)]
    cnt_ab = np.zeros((NC, 4), np.int64)
    for core in range(NC):
        cnt_ab[core] = np.bincount(am[arc_ids_core[core]] // 128, minlength=4)
    arc_tpb = -(-cnt_ab.max(axis=0) // 128)        # tiles per bucket
    arc_buckets = [b for b in range(4) for _ in range(arc_tpb[b])]
    if len(arc_buckets) % 2:
        arc_buckets.append(0)
        arc_tpb = arc_tpb.copy()
        arc_pad0 = 1
    else:
        arc_pad0 = 0
    n_arc_tile = len(arc_buckets)
    arc_tile_off = np.zeros(5, np.int64)
    np.cumsum(arc_tpb, out=arc_tile_off[1:5])

    in_maps = []
    meta = {"arc_slots": [], "sib_slots": [],
            "sib_combos": sib_combos, "arc_buckets": arc_buckets}
    for core in range(NC):
        m = dict(base)
        # per-core D window (transposed): cols [448-64c, 1023-64c), zero-pad to 576
        win = np.zeros((512, 576), f32)
        win[:, :575] = DfullT[:, 448 - 64 * core:1023 - 64 * core]
        m["dwin_in"] = bf(win.reshape(4, 128, 576).transpose(1, 0, 2))
        hsel = np.zeros((512, 64), f32)
        hsel[64 * core + np.arange(64), np.arange(64)] = 1.0
        m["hsel_in"] = bf(hsel.reshape(4, 128, 64).transpose(1, 0, 2))

        # arcs owned by this core (h in [64c, 64c+64))
        ids = arc_ids_core[core]
        mb = am[ids] // 128
        arc_slot = np.full(n_arc_tile * 128, -1, np.int64)
        order_a = np.argsort(mb, kind="stable")
        pos = 0
        for b in range(4):
            n = cnt_ab[core][b]
            s0 = arc_tile_off[b] * 128
            arc_slot[s0:s0 + n] = ids[order_a[pos:pos + n]]
            pos += n
        arc_rows = np.zeros((n_arc_tile, 128), np.int64)
        arc_mcol = np.zeros((128, n_arc_tile), f32)
        for t in range(n_arc_tile):
            sel = arc_slot[t * 128:(t + 1) * 128]
            valid = sel >= 0
            arc_rows[t, valid] = ah[sel[valid]] - 64 * core
            arc_mcol[valid, t] = am[sel[valid]] - 128 * arc_buckets[t]
        aoh = np.zeros((n_arc_tile // 2, 64, 256), _BF)
        avals = arc_rows.reshape(n_arc_tile // 2, 2, 128)
        acols = (np.arange(2)[:, None] * 128 + np.arange(128)[None, :])
        aoh[np.arange(n_arc_tile // 2)[:, None, None], avals, acols[None]] = 1
        m["arc_oh_in"] = aoh
        m["arcm_in"] = arc_mcol
        meta["arc_slots"].append(arc_slot)

        # sibs: this core's share of each combo, packed into the combo's tiles
        sib_slot = np.full(n_sib_tile * 128, -1, np.int64)
        for c in range(64):
            gids = sib_ids_by_combo[c]
            n = len(gids)
            base_n, rem = divmod(n, NC)
            lo = core * base_n + min(core, rem)
            hi = lo + base_n + (1 if core < rem else 0)
            part = gids[lo:hi]
            s0 = sib_tile_off[c] * 128
            assert len(part) <= sib_tpc[c] * 128
            sib_slot[s0:s0 + len(part)] = part
        idx_rows = np.zeros((n_sib_tile, 3, 128), np.int64)
        for t in range(n_sib_tile):
            c = sib_combos[t]
            hc, mc_, sc_ = c // 16, (c // 4) % 4, c % 4
            sel = sib_slot[t * 128:(t + 1) * 128]
            valid = sel >= 0
            sv = np.where(valid, sel, 0)
            idx_rows[t, 0] = np.where(valid, sh_i[sv] - 128 * hc, 0)
            idx_rows[t, 1] = np.where(valid, sm_i[sv] - 128 * mc_, 0)
            idx_rows[t, 2] = np.where(valid, ss_i[sv] - 128 * sc_, 0)
        assert idx_rows.max() < 128 and idx_rows.min() >= 0
        soh = np.zeros((n_sib_tile // 2, 128, 768), _BF)
        svals = idx_rows.reshape(n_sib_tile // 2, 2, 3, 128)
        scols = (np.arange(2)[:, None, None] * 384
                 + np.arange(3)[None, :, None] * 128
                 + np.arange(128)[None, None, :])
        soh[np.arange(n_sib_tile // 2)[:, None, None, None], svals,
            scols[None]] = 1
        # padding tile shares a real combo's one-hot slot: zero it out
        if sib_pad:
            soh[-1, :, 384:768] = 0
        m["sib_oh_in"] = soh
        meta["sib_slots"].append(sib_slot)
        in_maps.append(m)
    return in_maps, meta


LAST_EXEC_NS = None


def kernel(**inputs):
    global LAST_EXEC_NS
    _install_ntff_hook()
    from concourse.bass_utils import run_bass_kernel_spmd

    in_maps, meta = _host_prepare(inputs)
    nc = _get_program(meta["sib_combos"], meta["arc_buckets"])
    import os

    trace = os.environ.get("KERNEL_TRACE", "0") == "1"
    res = run_bass_kernel_spmd(nc, in_maps, list(range(NC)), trace=trace)
    LAST_EXEC_NS = res.exec_time_ns
    _CACHE["res"] = res
    n_sib_tile = len(meta["sib_combos"])
    arc_scores = np.zeros(A, np.float32)
    sib_scores = np.zeros(ASIB, np.float32)
    for core in range(NC):
        sc = np.asarray(res.results[core]["scores_out"])  # [128, n_tile]
        sib_flat = sc[:, :n_sib_tile].T.reshape(-1)
        sib_slot = meta["sib_slots"][core]                # global sib ids
        valid = sib_slot >= 0
        sib_scores[sib_slot[valid]] = sib_flat[valid]

        arc_flat = sc[:, n_sib_tile:].T.reshape(-1)
        arc_slot = meta["arc_slots"][core]                # global arc ids
        valid = arc_slot >= 0
        arc_scores[arc_slot[valid]] = arc_flat[valid]
    return np.concatenate([arc_scores, sib_scores])



# revision 31
# speedup vs baseline: 1.0733x; 1.0733x over previous
"""Trainium2 Bass kernel for nn_DependencyNeuralModel (dependency parser scorer).

v2 design (8 NeuronCores, SPMD):
  Encoder: 2-layer BiLSTM over S=512, replicated on every core, chunk-parallel
    (64 chunks x 2 dirs advance lock-step as 128 rows through the PE).
    K_WARM=16 warmup steps; gate order repacked to [i,f,o,g] so the i/f
    half of the recurrent GEMM can overlap the o/g half's activations.
  Arc scores: score(h,m) depends only on the (h,m) pair (dist is a function
    of m-h), so each core builds the 64-row slice of the full SxS score
    table it owns (h sharded), in a transposed layout where the dist term
    is a contiguous slice of a host-built [H, 1023] offset table and the
    head term is a per-partition activation-fused bias.  The per-arc gather
    is then a single GPSIMD ap_gather of scalar (pair) entries from the
    partition-replicated table; host picks the parity lane and unsorts.
  Sib scores: part-sharded; host sorts each core's 16384 parts by the
    (head,mod,sib) 128-chunk combo (64 combos x 3 static tiles), so each
    128-part tile needs only 3 one-hot gather matmuls instead of 12.
Host does only index/layout preparation and final unshard.
"""
import sys
import types

import numpy as np

sys.path.insert(0, "/opt/trn_rl_repo")

import concourse.bass as bass
import concourse.mybir as mybir
from concourse.tile import TileContext
from concourse.masks import make_identity

S = 512
H = 512
A = 262144
ASIB = 131072
NB = 17
L = 8
K_WARM = 16
NSTEP = K_WARM + L  # 24
NC = 8
F32 = mybir.dt.float32
BF16 = mybir.dt.bfloat16
U16 = mybir.dt.uint16
BINS = np.array(list(range(10)) + list(range(10, 40, 5)) + [40], dtype=np.int64)

GPERM = np.r_[0:1024, 1536:2048, 1024:1536]  # gate reorder i,f,g,o -> i,f,o,g


def _install_ntff_hook():
    if "antenv.axon_hooks" in sys.modules:
        return
    mod = types.ModuleType("antenv.axon_hooks")
    state = {"hook": None, "tried": False}

    def set_axon_ntff_profile_hook(hook):
        state["hook"] = hook

    def get_axon_ntff_profile_hook():
        if state["hook"] is None and not state["tried"]:
            state["tried"] = True
            try:
                from trn_agent_boot.trn_boot import _ntff_profile_via_ctypes

                state["hook"] = _ntff_profile_via_ctypes("/opt/axon/libaxon_pjrt.so")
            except Exception:
                state["hook"] = None
        return state["hook"]

    mod.set_axon_ntff_profile_hook = set_axon_ntff_profile_hook
    mod.get_axon_ntff_profile_hook = get_axon_ntff_profile_hook
    import antenv

    antenv.axon_hooks = mod
    sys.modules["antenv.axon_hooks"] = mod


def _legalize_waits(nc):
    """This walrus accepts at most one semaphore wait per instruction;
    split extra waits onto same-engine NOPs placed just before."""
    ctr = [0]
    for f in nc.m.functions:
        for blk in f.blocks:
            out = []
            dirty = False
            for ins in blk.instructions:
                si = ins.sync_info
                if si is not None and si.on_wait and len(si.on_wait) > 1:
                    waits = list(si.on_wait)
                    for w in waits[:-1]:
                        ctr[0] += 1
                        nop = mybir.InstNoOp(name=f"waitfix-{ctr[0]}")
                        nop.engine = ins.engine
                        nop.sync_info = mybir.SyncInfo(on_wait=[w], on_update=[])
                        out.append(nop)
                    ins.sync_info = mybir.SyncInfo(
                        on_wait=[waits[-1]],
                        on_update=list(si.on_update) if si.on_update else [],
                    )
                    dirty = True
                out.append(ins)
            if dirty:
                blk.instructions = out
    return nc


def _lstm_layer(nc, tc, ident, mask_sb, whhT_dram, wx_dram, f_dram, b_dram):
    """One BiLSTM layer, chunk-parallel.  B=128 rows: partitions 0:64 are
    dir0 chunks, 64:128 dir1 chunks.  Gate columns are [i,f,o,g]; the
    i/f half of the recurrent GEMM is emitted first so its activations
    overlap the o/g half's matmuls."""
    import contextlib

    with contextlib.ExitStack() as ctx:
        sb = ctx.enter_context(tc.tile_pool(name="lstm_sb", bufs=3))
        cold = ctx.enter_context(tc.tile_pool(name="lstm_cold", bufs=1))
        st = ctx.enter_context(tc.tile_pool(name="lstm_state", bufs=1))
        ps1 = ctx.enter_context(tc.tile_pool(name="lstm_ps1", bufs=2, space="PSUM"))
        ps2 = ctx.enter_context(tc.tile_pool(name="lstm_ps2", bufs=1, space="PSUM"))
        pst = ctx.enter_context(tc.tile_pool(name="lstm_pst", bufs=1, space="PSUM"))

        whh_sb = st.tile([128, 4, 2, 2048], BF16)
        nc.sync.dma_start(
            whh_sb.rearrange("p a b c -> p (a b c)"),
            whhT_dram.rearrange("p a b c -> p (a b c)"),
        )
        h_t = st.tile([128, 4, 128], BF16)  # h transposed: [k-part, kc, b]
        c_st = st.tile([128, 512], BF16)    # [b, k]
        nc.vector.memset(h_t.rearrange("p a b -> p (a b)"), 0.0)
        nc.vector.memset(c_st[:], 0.0)

        def emit_wx_ident(s):
            """wx DMA + identity-injection matmuls for step s; emitted during
            step s-1's tail so the PE fills otherwise-idle cycles."""
            wx = sb.tile([128, 2048], BF16, tag="wx")
            for d in range(2):
                nc.sync.dma_start(
                    wx[d * 64:(d + 1) * 64, :], wx_dram[d, s:s + 505:8, :]
                )
            g01 = ps1.tile([128, 1024], F32, tag="g01")
            g23 = ps2.tile([128, 1024], F32, tag="g23")
            for half, gps in ((0, g01), (1, g23)):
                for ng in range(2):
                    col = (half * 2 + ng) * 512
                    for d in range(2):
                        bs = slice(d * 64, (d + 1) * 64)
                        nc.tensor.matmul(
                            gps[bs, ng * 512:(ng + 1) * 512],
                            lhsT=ident[:, bs],
                            rhs=wx[:, col:col + 512],
                            start=True, stop=False,
                        )
            return g01, g23

        nxt = emit_wx_ident(0)
        for s in range(NSTEP):
            # recurrent half of the gates GEMM.  d0 writes psum rows 0:64
            # (array cols 0-63), d1 rows 64:128 (cols 64-127); adjacent
            # d0/d1 matmuls run concurrently as (128,64) column tiles.
            g01, g23 = nxt
            for half, gps in ((0, g01), (1, g23)):
                for ng in range(2):
                    col = (half * 2 + ng) * 512
                    for kc in range(4):
                        for d in range(2):
                            bs = slice(d * 64, (d + 1) * 64)
                            nc.tensor.matmul(
                                gps[bs, ng * 512:(ng + 1) * 512],
                                lhsT=h_t[:, kc, bs],
                                rhs=whh_sb[:, kc, d, col:col + 512],
                                start=False,
                                stop=(kc == 3),
                            )
            sig_if = cold.tile([128, 1024], BF16, tag="sif")
            nc.scalar.activation(sig_if[:], g01[:],
                                 mybir.ActivationFunctionType.Sigmoid)
            tanh_g = cold.tile([128, 512], BF16, tag="tg")
            nc.scalar.activation(tanh_g[:], g23[:, 512:1024],
                                 mybir.ActivationFunctionType.Tanh)
            sig_o = cold.tile([128, 512], BF16, tag="so")
            nc.scalar.activation(sig_o[:], g23[:, 0:512],
                                 mybir.ActivationFunctionType.Sigmoid)
            t1 = cold.tile([128, 512], BF16, tag="t1")
            nc.vector.tensor_mul(t1[:], sig_if[:, 512:1024], c_st[:])
            t2 = cold.tile([128, 512], BF16, tag="t2")
            nc.vector.tensor_mul(t2[:], sig_if[:, 0:512], tanh_g[:])
            nc.vector.tensor_add(c_st[:], t1[:], t2[:])
            tch = cold.tile([128, 512], BF16, tag="tch")
            nc.scalar.activation(tch[:], c_st[:], mybir.ActivationFunctionType.Tanh)
            h_new = cold.tile([128, 512], BF16, tag="h")
            nc.vector.tensor_mul(h_new[:], sig_o[:], tch[:])
            if s in (7, 15) and s < K_WARM:
                mi = {7: 0, 15: 1}[s]
                nc.vector.tensor_scalar_mul(h_new[:], h_new[:], mask_sb[:, mi:mi + 1])
                nc.vector.tensor_scalar_mul(c_st[:], c_st[:], mask_sb[:, mi:mi + 1])
            if s + 1 < NSTEP:
                # next step's wx/identity matmuls go into the PE queue BEFORE
                # this step's transposes: they have no h dependency and run
                # while the tail above executes.
                nxt = emit_wx_ident(s + 1)
            tp = pst.tile([128, 4, 128], BF16, tag="tr_ps")
            for kc in range(4):
                nc.tensor.transpose(tp[:, kc, :],
                                    h_new[:, kc * 128:(kc + 1) * 128], ident[:])
            nc.vector.tensor_copy(h_t.rearrange("p a b -> p (a b)"),
                                  tp.rearrange("p a b -> p (a b)"))
            if s >= K_WARM:
                o = s - K_WARM
                nc.sync.dma_start(f_dram[o:505 + o:8, :], h_new[0:64, :])
                nc.sync.dma_start(b_dram[o:505 + o:8, :], h_new[64:128, :])


def _transpose_pair(nc, tc, ident, rev, f_dram, b_dram, dstT, dstTrev, one_row):
    """Build [feat, pos] lhsT chunks (and optionally pos-reversed copy) from
    the per-direction output buffers.  dstT/dstTrev: [128, 9, 512] tiles;
    chunk 8 row 0 is set to ones (bias); rest of chunk 8 zero."""
    import contextlib

    with contextlib.ExitStack() as ctx:
        sb = ctx.enter_context(tc.tile_pool(name="tp_sb", bufs=3))
        ps = ctx.enter_context(tc.tile_pool(name="tp_ps", bufs=2, space="PSUM"))
        for dst in (dstT, dstTrev):
            if dst is None:
                continue
            nc.vector.memset(dst[:, 8, :], 0.0)
            nc.vector.tensor_copy(dst[0:1, 8, :], one_row[:])
        for j in range(4):
            for pc in range(4):
                fsrc = sb.tile([128, 128], BF16, tag="fsrc")
                nc.sync.dma_start(fsrc[:], f_dram[pc * 128:(pc + 1) * 128,
                                                  j * 128:(j + 1) * 128])
                tp = ps.tile([128, 128], BF16, tag="tp")
                nc.tensor.transpose(tp[:], fsrc[:], ident[:])
                nc.vector.tensor_copy(dstT[:, j, pc * 128:(pc + 1) * 128], tp[:])
                if dstTrev is not None:
                    tpr = ps.tile([128, 128], BF16, tag="tpr")
                    nc.tensor.transpose(tpr[:], fsrc[:], rev[:])
                    nc.vector.tensor_copy(
                        dstTrev[:, j, (3 - pc) * 128:(4 - pc) * 128], tpr[:])
                bsrc = sb.tile([128, 128], BF16, tag="bsrc")
                nc.sync.dma_start(bsrc[:], b_dram[pc * 128:(pc + 1) * 128,
                                                  j * 128:(j + 1) * 128])
                # b rows are scan order q; position = 511-q: reverse via rev
                tpb = ps.tile([128, 128], BF16, tag="tpb")
                nc.tensor.transpose(tpb[:], bsrc[:], rev[:])
                nc.vector.tensor_copy(
                    dstT[:, 4 + j, (3 - pc) * 128:(4 - pc) * 128], tpb[:])
                if dstTrev is not None:
                    tpb2 = ps.tile([128, 128], BF16, tag="tpb2")
                    nc.tensor.transpose(tpb2[:], bsrc[:], ident[:])
                    nc.vector.tensor_copy(
                        dstTrev[:, 4 + j, pc * 128:(pc + 1) * 128], tpb2[:])


def _input_gemm(nc, tc, lhsT_tiles, wihT_dram, wx_dram, nk, klast):
    """WX[d] = lhsT_d.T @ wihT[d] -> wx_dram[d, K_WARM:K_WARM+512, :].
    lhsT_tiles: per-dir tile [128, nk, 512] in SBUF ([feat-part, chunk, pos]).
    nk chunks; last chunk has klast valid rows."""
    import contextlib

    with contextlib.ExitStack() as ctx:
        sb = ctx.enter_context(tc.tile_pool(name="ig_sb", bufs=6))
        ps = ctx.enter_context(tc.tile_pool(name="ig_ps", bufs=2, space="PSUM"))
        for d in range(2):
            lhsT = lhsT_tiles[d]
            for ngc in range(4):
                acc4 = ps.tile([128, 4, 512], F32, tag="acc4")
                for kc in range(nk):
                    kk = 128 if kc < nk - 1 else klast
                    rhs = sb.tile([128, 512], wihT_dram.dtype, tag="rhs")
                    nc.sync.dma_start(
                        rhs[:kk, :],
                        wihT_dram[kc * 128:kc * 128 + kk, d,
                                  ngc * 512:(ngc + 1) * 512],
                    )
                    for mc in range(4):
                        nc.tensor.matmul(
                            acc4[:, mc, :],
                            lhsT=lhsT[:kk, kc, mc * 128:(mc + 1) * 128],
                            rhs=rhs[:kk, :],
                            start=(kc == 0),
                            stop=(kc == nk - 1),
                        )
                osb = sb.tile([128, 4, 512], BF16, tag="osb")
                nc.scalar.activation(
                    osb.rearrange("p a b -> p (a b)"),
                    acc4.rearrange("p a b -> p (a b)"),
                    mybir.ActivationFunctionType.Copy)
                for mc in range(4):
                    nc.sync.dma_start(
                        wx_dram[d, K_WARM + mc * 128:K_WARM + (mc + 1) * 128,
                                ngc * 512:(ngc + 1) * 512],
                        osb[:, mc, :],
                    )


def _build(nc, sib_combos, arc_buckets):
    dt = F32
    n_sib_tile = len(sib_combos)       # even
    n_arc_tile = len(arc_buckets)      # even
    n_tile = n_sib_tile + n_arc_tile
    embT_f = nc.dram_tensor("embT_f", [128, 3, 512], BF16, kind="ExternalInput")
    embT_b = nc.dram_tensor("embT_b", [128, 3, 512], BF16, kind="ExternalInput")
    wih0T = nc.dram_tensor("wih0T", [384, 2, 2048], BF16, kind="ExternalInput")
    whh0T = nc.dram_tensor("whh0T", [128, 4, 2, 2048], BF16, kind="ExternalInput")
    wih1T = nc.dram_tensor("wih1T", [1152, 2, 2048], BF16, kind="ExternalInput")
    whh1T = nc.dram_tensor("whh1T", [128, 4, 2, 2048], BF16, kind="ExternalInput")
    projT = nc.dram_tensor("projT", [1152, 2560], BF16, kind="ExternalInput")
    dwin_in = nc.dram_tensor("dwin_in", [128, 4, 576], BF16, kind="ExternalInput")
    hsel_in = nc.dram_tensor("hsel_in", [128, 4, 64], BF16, kind="ExternalInput")
    wrep_in = nc.dram_tensor("wrep_in", [128, 512], BF16, kind="ExternalInput")
    wrepT_in = nc.dram_tensor("wrepT_in", [128, 4, 128], BF16, kind="ExternalInput")
    sib_oh_in = nc.dram_tensor("sib_oh_in", [n_sib_tile // 2, 128, 768], BF16,
                               kind="ExternalInput")
    arc_oh_in = nc.dram_tensor("arc_oh_in", [n_arc_tile // 2, 64, 256], dt,
                               kind="ExternalInput")
    arcm_in = nc.dram_tensor("arcm_in", [128, n_arc_tile], dt,
                             kind="ExternalInput")
    iotar_in = nc.dram_tensor("iotar_in", [128, 128], dt, kind="ExternalInput")
    mask_in = nc.dram_tensor("mask_in", [128, 2], dt, kind="ExternalInput")
    rev_in = nc.dram_tensor("rev_in", [128, 128], BF16, kind="ExternalInput")
    scores_out = nc.dram_tensor("scores_out", [128, n_tile], dt,
                                kind="ExternalOutput")

    wx0 = nc.dram_tensor("wx0", [2, 544, 2048], BF16)
    tdram = nc.dram_tensor("tdram", [64, 512], F32)
    wx1 = nc.dram_tensor("wx1", [2, 544, 2048], BF16)
    tdram = nc.dram_tensor("tdram", [64, 512], F32)
    f0d = nc.dram_tensor("f0d", [512, 512], BF16)
    b0d = nc.dram_tensor("b0d", [512, 512], BF16)
    f1d = nc.dram_tensor("f1d", [512, 512], BF16)
    b1d = nc.dram_tensor("b1d", [512, 512], BF16)

    import contextlib

    with TileContext(nc) as tc:
        with contextlib.ExitStack() as ctx:
            const = ctx.enter_context(tc.tile_pool(name="const", bufs=1))
            big = ctx.enter_context(tc.tile_pool(name="big", bufs=1))

            ident = const.tile([128, 128], BF16)
            make_identity(nc, ident[:])
            rev = const.tile([128, 128], BF16)
            nc.sync.dma_start(rev[:], rev_in[:])
            mask_sb = const.tile([128, 2], dt)
            nc.sync.dma_start(mask_sb[:], mask_in[:])
            one_row = const.tile([1, 512], BF16)
            nc.vector.memset(one_row[:], 1.0)
            wrep_sb = const.tile([128, 512], BF16)
            nc.sync.dma_start(wrep_sb[:], wrep_in[:])
            wrepT_sb = const.tile([128, 4, 128], BF16)
            nc.sync.dma_start(wrepT_sb.rearrange("p a b -> p (a b)"),
                              wrepT_in.rearrange("p a b -> p (a b)"))
            dwin_sb = const.tile([128, 4, 576], BF16)
            nc.sync.dma_start(dwin_sb.rearrange("p a b -> p (a b)"),
                              dwin_in.rearrange("p a b -> p (a b)"))
            hsel_sb = const.tile([128, 4, 64], BF16)
            nc.sync.dma_start(hsel_sb.rearrange("p a b -> p (a b)"),
                              hsel_in.rearrange("p a b -> p (a b)"))
            iota_row = const.tile([128, 128], dt)
            nc.sync.dma_start(iota_row[:], iotar_in[:])
            arcm_sb = const.tile([128, n_arc_tile], dt)
            nc.sync.dma_start(arcm_sb[:], arcm_in[:])

            # zero-pad warmup rows of WX buffers
            with tc.tile_pool(name="zp", bufs=1) as zp:
                zrow = zp.tile([64, 2048], BF16)
                nc.vector.memset(zrow[:], 0.0)
                for wxd in (wx0, wx1):
                    for d in range(2):
                        nc.sync.dma_start(wxd[d, 0:K_WARM, :], zrow[0:K_WARM, :])
                        nc.sync.dma_start(wxd[d, K_WARM + 512:544, :],
                                          zrow[0:32 - K_WARM, :])

            # ---- WX0 ----
            with tc.tile_pool(name="emb_sb", bufs=1) as emb_pool:
                ef = emb_pool.tile([128, 3, 512], BF16)
                nc.sync.dma_start(ef.rearrange("p a b -> p (a b)"),
                                  embT_f.rearrange("p a b -> p (a b)"))
                eb = emb_pool.tile([128, 3, 512], BF16)
                nc.sync.dma_start(eb.rearrange("p a b -> p (a b)"),
                                  embT_b.rearrange("p a b -> p (a b)"))
                _input_gemm(nc, tc, [ef, eb], wih0T, wx0, 3, 128)

            # ---- layer 0 ----
            _lstm_layer(nc, tc, ident, mask_sb, whh0T, wx0, f0d, b0d)

            # ---- x1T / x1Trev ----
            x1T = big.tile([128, 9, 512], BF16, tag="x1T")
            x1Trev = big.tile([128, 9, 512], BF16, tag="x1Trev")
            _transpose_pair(nc, tc, ident, rev, f0d, b0d, x1T, x1Trev, one_row)

            # ---- WX1 ----
            _input_gemm(nc, tc, [x1T, x1Trev], wih1T, wx1, 9, 1)

            # ---- layer 1 ----
            _lstm_layer(nc, tc, ident, mask_sb, whh1T, wx1, f1d, b1d)

            # ---- statesT ----
            stT = big.tile([128, 9, 512], BF16, tag="x1T")  # reuse x1T slot
            _transpose_pair(nc, tc, ident, rev, f1d, b1d, stT, None, one_row)

            # ---- pos-major projection tables (head + 3 sib; skip mod) ----
            tables_sb = big.tile([128, 4, 2560], BF16, tag="tables")
            with contextlib.ExitStack() as c2:
                sb2 = c2.enter_context(tc.tile_pool(name="tb_sb", bufs=6))
                with tc.tile_pool(name="tb_ps4", bufs=2, space="PSUM") as ps4:
                    for ngc in (0, 2, 3, 4):
                        acc4 = ps4.tile([128, 4, 512], dt, tag="acc4")
                        for kc in range(9):
                            kk = 128 if kc < 8 else 1
                            rhs = sb2.tile([128, 512], BF16, tag="rhs")
                            nc.sync.dma_start(
                                rhs[:kk, :],
                                projT[kc * 128:kc * 128 + kk,
                                      ngc * 512:(ngc + 1) * 512],
                            )
                            for mc in range(4):
                                nc.tensor.matmul(
                                    acc4[:, mc, :],
                                    lhsT=stT[:kk, kc, mc * 128:(mc + 1) * 128],
                                    rhs=rhs[:kk, :],
                                    start=(kc == 0),
                                    stop=(kc == 8),
                                )
                        for mc in range(4):
                            nc.scalar.activation(
                                tables_sb[:, mc, ngc * 512:(ngc + 1) * 512],
                                acc4[:, mc, :],
                                mybir.ActivationFunctionType.Copy)
                ps2 = c2.enter_context(tc.tile_pool(name="tb_ps", bufs=2,
                                                    space="PSUM"))

                # ---- transposed mod table M_T[j, m] ----
                mT = big.tile([128, 4, 512], BF16, tag="mT")
                for jc in range(4):
                    acc = ps2.tile([128, 512], dt, tag="acc")
                    for kc in range(8):
                        lh = sb2.tile([128, 128], BF16, tag="lh")
                        nc.sync.dma_start(
                            lh[:],
                            projT[kc * 128:(kc + 1) * 128,
                                  512 + jc * 128:512 + (jc + 1) * 128],
                        )
                        nc.tensor.matmul(
                            acc[:], lhsT=lh[:], rhs=stT[:, kc, :],
                            start=(kc == 0), stop=(kc == 7),
                        )
                    nc.scalar.activation(mT[:, jc, :], acc[:],
                                         mybir.ActivationFunctionType.Copy)

                # ---- H window: hwin[j, hl] = heads[64c+hl, j] ----
                hwin = big.tile([128, 4, 64], dt, tag="hwin")
                for jc in range(4):
                    acc = ps2.tile([128, 64], dt, tag="acch")
                    for kc in range(4):
                        nc.tensor.matmul(
                            acc[:],
                            lhsT=tables_sb[:, kc, jc * 128:(jc + 1) * 128],
                            rhs=hsel_sb[:, kc, :],
                            start=(kc == 0), stop=(kc == 3),
                        )
                    nc.vector.tensor_copy(hwin[:, jc, :], acc[:])

            # ---- scoring ----
            scores_sb = big.tile([128, n_tile], dt, tag="scores")
            with contextlib.ExitStack() as c3:
                sb3 = c3.enter_context(tc.tile_pool(name="sc_sb", bufs=4))
                ps_sacc = c3.enter_context(tc.tile_pool(name="ps_sacc", bufs=2,
                                                        space="PSUM"))

                def sib_pair(p):
                    # sib tiles 2p, 2p+1 with host-uploaded one-hots
                    oh = sb3.tile([128, 2, 3, 128], BF16, tag="oh")
                    nc.sync.dma_start(
                        oh.rearrange("p a b c -> p (a b c)"),
                        sib_oh_in[p, :, :],
                    )
                    sacc2 = ps_sacc.tile([128, 2, 512], dt, tag="sacc2")
                    for h2 in range(2):
                        combo = sib_combos[2 * p + h2]
                        chunks = (combo // 16, (combo // 4) % 4, combo % 4)
                        for g in range(3):
                            nc.tensor.matmul(
                                sacc2[:, h2, :], lhsT=oh[:, h2, g, :],
                                rhs=tables_sb[:, chunks[g],
                                              1024 + g * 512:
                                              1024 + (g + 1) * 512],
                                start=(g == 0), stop=(g == 2),
                            )
                    th2 = sb3.tile([128, 2, 512], BF16, tag="th2")
                    nc.scalar.activation(
                        th2.rearrange("p a b -> p (a b)"),
                        sacc2.rearrange("p a b -> p (a b)"),
                        mybir.ActivationFunctionType.Tanh)
                    for h2 in range(2):
                        junk = sb3.tile([128, 512], BF16, tag="junk")
                        nc.vector.scalar_tensor_tensor(
                            junk[:], th2[:, h2, :], 1.0, wrep_sb[:],
                            op0=mybir.AluOpType.mult,
                            op1=mybir.AluOpType.mult,
                            accum_out=scores_sb[:, 2 * p + h2:2 * p + h2 + 1],
                        )

                N_PAIR_A = 16
                # phase A: table rows interleaved with first sib pairs
                with tc.tile_pool(name="ps_tblw", bufs=2, space="PSUM") as ps_w:
                    for hl in range(64):
                        tmp = sb3.tile([128, 4, 512], BF16, tag="tmp")
                        nc.vector.tensor_add(
                            tmp[:], mT[:],
                            dwin_sb[:, :, 63 - hl:63 - hl + 512])
                        tht = sb3.tile([128, 4, 512], BF16, tag="tht")
                        for jc in range(4):
                            nc.scalar.activation(
                                tht[:, jc, :], tmp[:, jc, :],
                                mybir.ActivationFunctionType.Tanh,
                                bias=hwin[:, jc, hl:hl + 1])
                        wps = ps_w.tile([128, 512], dt, tag="wps")
                        for jc in range(4):
                            nc.tensor.matmul(
                                wps[:], lhsT=wrepT_sb[:, jc, :],
                                rhs=tht[:, jc, :],
                                start=(jc == 0), stop=(jc == 3),
                            )
                        trow = sb3.tile([128, 512], dt, tag="trow")
                        nc.vector.tensor_copy(trow[:], wps[:])
                        nc.sync.dma_start(tdram[hl:hl + 1, :], trow[0:1, :])
                        if hl < N_PAIR_A:
                            sib_pair(hl)

                table_hm = big.tile([64, 512], dt, tag="table_hm")
                nc.sync.dma_start(table_hm[:], tdram[:])

                # phase B: remaining sib pairs + arc gather tile pairs
                with tc.tile_pool(name="ps_arc", bufs=2, space="PSUM") as ps_a:

                    def arc_pair(pa):
                        # arc tiles u=2pa, 2pa+1 with host-uploaded h one-hots
                        u = 2 * pa
                        ohh = sb3.tile([64, 2, 128], dt, tag="ohh")
                        nc.sync.dma_start(
                            ohh.rearrange("p a b -> p (a b)"),
                            arc_oh_in[pa, :, :],
                        )
                        comb = ps_a.tile([128, 2, 128], dt, tag="comb")
                        for h2 in range(2):
                            bucket = arc_buckets[u + h2]
                            nc.tensor.matmul(
                                comb[:, h2, :], lhsT=ohh[:, h2, :],
                                rhs=table_hm[0:64,
                                             bucket * 128:(bucket + 1) * 128],
                                start=True, stop=True,
                            )
                        for h2 in range(2):
                            junk2 = sb3.tile([128, 128], BF16, tag="junk2")
                            nc.vector.scalar_tensor_tensor(
                                junk2[:], iota_row[:],
                                arcm_sb[:, u + h2:u + h2 + 1],
                                comb[:, h2, :],
                                op0=mybir.AluOpType.is_equal,
                                op1=mybir.AluOpType.mult,
                                accum_out=scores_sb[:, n_sib_tile + u + h2:
                                                    n_sib_tile + u + h2 + 1],
                            )

                    n_sib_pair = n_sib_tile // 2
                    n_arc_pair = n_arc_tile // 2
                    nxt = 0
                    nb = n_sib_pair - N_PAIR_A
                    for k in range(nb):
                        sib_pair(N_PAIR_A + k)
                        na = (n_arc_pair * (k + 1)) // nb - (n_arc_pair * k) // nb
                        for _ in range(na):
                            if nxt < n_arc_pair:
                                arc_pair(nxt)
                                nxt += 1
                    while nxt < n_arc_pair:
                        arc_pair(nxt)
                        nxt += 1

                nc.sync.dma_start(scores_out[:], scores_sb[:])
    return nc


_CACHE = {}


def _get_program(sib_combos, arc_buckets):
    key = (tuple(sib_combos), tuple(arc_buckets))
    if _CACHE.get("key") != key:
        nc = bass.Bass()
        _build(nc, sib_combos, arc_buckets)
        _legalize_waits(nc)
        _CACHE["nc"] = nc
        _CACHE["key"] = key
    return _CACHE["nc"]


def _host_prepare(inputs):
    import jax.numpy as jnp
    import ml_dtypes
    _BF = ml_dtypes.bfloat16

    def bf(x):
        return np.asarray(jnp.asarray(np.asarray(x, np.float32), jnp.bfloat16))

    f32 = np.float32
    words = np.asarray(inputs["words"]).astype(np.int64)
    tags = np.asarray(inputs["tags"]).astype(np.int64)
    word_emb = np.asarray(inputs["word_emb"], f32)
    tag_emb = np.asarray(inputs["tag_emb"], f32)
    emb = np.concatenate([word_emb[words], tag_emb[tags]], axis=-1)  # [512, 364]
    emb_aug = np.concatenate([emb, np.ones((S, 1), f32)], axis=1)    # [512, 365]

    def packT(x, rows):  # -> [rows(pad), ...] = x.T zero-padded
        out = np.zeros((rows, x.shape[0]), f32)
        out[: x.shape[1]] = x.T
        return out

    embT_f = bf(packT(emb_aug, 384).reshape(3, 128, 512).transpose(1, 0, 2))
    embT_b = bf(packT(emb_aug[::-1], 384).reshape(3, 128, 512).transpose(1, 0, 2))

    def wih_pack(Wih, bih, bhh, kdim, rows):
        out = np.zeros((rows, 2, 4 * H), f32)
        for d in range(2):
            out[:kdim, d] = np.asarray(Wih[d], f32).T[:, GPERM]
            out[kdim, d] = (np.asarray(bih[d], f32) + np.asarray(bhh[d], f32))[GPERM]
        return out

    wih0T = bf(wih_pack(inputs["Wih0"], inputs["bih0"], inputs["bhh0"], 364, 384))
    wih1T = bf(wih_pack(inputs["Wih1"], inputs["bih1"], inputs["bhh1"], 1024, 1152))

    def whh_pack(Whh):
        out = np.zeros((128, 4, 2, 4 * H), f32)
        for d in range(2):
            wt = np.asarray(Whh[d], f32).T[:, GPERM]  # [512 k, 2048 g]
            out[:, :, d, :] = wt.reshape(4, 128, 4 * H).transpose(1, 0, 2)
        return out

    whh0T = bf(whh_pack(inputs["Whh0"]))
    whh1T = bf(whh_pack(inputs["Whh1"]))

    projs = [inputs["head_W"], inputs["mod_W"], inputs["sib_head_W"],
             inputs["sib_mod_W"], inputs["sib_sib_W"]]
    projT = np.zeros((1152, 5 * H), f32)
    for i, W in enumerate(projs):
        projT[:1024, i * H:(i + 1) * H] = np.asarray(W, f32).T
    projT = bf(projT)

    w = np.asarray(inputs["arc_w"], f32).reshape(512)
    wrep = bf(np.broadcast_to(w, (128, 512)))
    wrepT = bf(w.reshape(4, 128).T.reshape(128, 4, 1).repeat(128, axis=2))

    # Dfull[off] = D[distidx(off - 511)], off in [0, 1022]
    D = (np.asarray(inputs["dist_emb"], f32) @ np.asarray(inputs["dist_W"], f32).T
         + np.asarray(inputs["dist_b"], f32))
    offs = np.arange(-511, 512)
    bi = np.searchsorted(BINS, np.abs(offs), side="right") - 1
    Dfull = D[np.where(offs > 0, bi, bi + NB)]          # [1023, H]
    DfullT = Dfull.T                                     # [H, 1023]

    iotar = np.tile(np.arange(128, dtype=f32), (128, 1))
    mask = np.zeros((128, 2), f32)
    for mi, s in enumerate((7, 15)):
        c = np.arange(64)
        v = ((8 * c + s) > (K_WARM - 1)).astype(f32)
        mask[0:64, mi] = v
        mask[64:128, mi] = v
    revm = np.zeros((128, 128), f32)
    revm[np.arange(128), 127 - np.arange(128)] = 1.0
    revm = bf(revm)

    base = {
        "embT_f": embT_f, "embT_b": embT_b,
        "wih0T": wih0T, "whh0T": whh0T, "wih1T": wih1T, "whh1T": whh1T,
        "projT": projT, "wrep_in": wrep, "wrepT_in": wrepT,
        "iotar_in": iotar, "mask_in": mask, "rev_in": revm,
    }

    ah = np.asarray(inputs["arc_head"]).astype(np.int64)
    am = np.asarray(inputs["arc_mod"]).astype(np.int64)
    sh_i = np.asarray(inputs["sib_head"]).astype(np.int64)
    sm_i = np.asarray(inputs["sib_mod"]).astype(np.int64)
    ss_i = np.asarray(inputs["sib_sib"]).astype(np.int64)

    # ---- global tile layouts (uniform across cores; program depends on them)
    # sibs are sharded BY COMBO: combo c's parts split evenly over cores, each
    # core gets ceil(cnt_g[c]/(128*NC)) tiles for combo c.
    combo_g = (sh_i // 128) * 16 + (sm_i // 128) * 4 + (ss_i // 128)
    cnt_g = np.bincount(combo_g, minlength=64)
    sib_tpc = -(-cnt_g // (128 * NC))              # tiles per combo per core
    sib_combos = [c for c in range(64) for _ in range(sib_tpc[c])]
    if len(sib_combos) % 2:
        sib_combos.append(int(np.argmax(sib_tpc == 0)) if (sib_tpc == 0).any()
                          else 0)
        sib_pad = 1
    else:
        sib_pad = 0
    n_sib_tile = len(sib_combos)
    sib_tile_off = np.zeros(65, np.int64)          # first tile of each combo
    np.cumsum(sib_tpc, out=sib_tile_off[1:65])
    sib_ids_by_combo = [np.nonzero(combo_g == c)[0] for c in range(64)]

    # arcs stay h-sharded (core owns a 64-row table slice); bucket tile counts
    # take the max over cores so the layout is core-uniform.
    core_of = ah // 64
    arc_ids_core = [np.nonzero(core_of == core)[0] for core in range(NC)]
    cnt_ab = np.zeros((NC, 4), np.int64)
    for core in range(NC):
        cnt_ab[core] = np.bincount(am[arc_ids_core[core]] // 128, minlength=4)
    arc_tpb = -(-cnt_ab.max(axis=0) // 128)        # tiles per bucket
    arc_buckets = [b for b in range(4) for _ in range(arc_tpb[b])]
    if len(arc_buckets) % 2:
        arc_buckets.append(0)
        arc_tpb = arc_tpb.copy()
        arc_pad0 = 1
    else:
        arc_pad0 = 0
    n_arc_tile = len(arc_buckets)
    arc_tile_off = np.zeros(5, np.int64)
    np.cumsum(arc_tpb, out=arc_tile_off[1:5])

    in_maps = []
    meta = {"arc_slots": [], "sib_slots": [],
            "sib_combos": sib_combos, "arc_buckets": arc_buckets}
    for core in range(NC):
        m = dict(base)
        # per-core D window (transposed): cols [448-64c, 1023-64c), zero-pad to 576
        win = np.zeros((512, 576), f32)
        win[:, :575] = DfullT[:, 448 - 64 * core:1023 - 64 * core]
        m["dwin_in"] = bf(win.reshape(4, 128, 576).transpose(1, 0, 2))
        hsel = np.zeros((512, 64), f32)
        hsel[64 * core + np.arange(64), np.arange(64)] = 1.0
        m["hsel_in"] = bf(hsel.reshape(4, 128, 64).transpose(1, 0, 2))

        # arcs owned by this core (h in [64c, 64c+64))
        ids = arc_ids_core[core]
        mb = am[ids] // 128
        arc_slot = np.full(n_arc_tile * 128, -1, np.int64)
        order_a = np.argsort(mb, kind="stable")
        pos = 0
        for b in range(4):
            n = cnt_ab[core][b]
            s0 = arc_tile_off[b] * 128
            arc_slot[s0:s0 + n] = ids[order_a[pos:pos + n]]
            pos += n
        arc_rows = np.zeros((n_arc_tile, 128), np.int64)
        arc_mcol = np.zeros((128, n_arc_tile), f32)
        for t in range(n_arc_tile):
            sel = arc_slot[t * 128:(t + 1) * 128]
            valid = sel >= 0
            arc_rows[t, valid] = ah[sel[valid]] - 64 * core
            arc_mcol[valid, t] = am[sel[valid]] - 128 * arc_buckets[t]
        aoh = np.zeros((n_arc_tile // 2, 64, 256), f32)
        avals = arc_rows.reshape(n_arc_tile // 2, 2, 128)
        acols = (np.arange(2)[:, None] * 128 + np.arange(128)[None, :])
        aoh[np.arange(n_arc_tile // 2)[:, None, None], avals, acols[None]] = 1
        m["arc_oh_in"] = aoh
        m["arcm_in"] = arc_mcol
        meta["arc_slots"].append(arc_slot)

        # sibs: this core's share of each combo, packed into the combo's tiles
        sib_slot = np.full(n_sib_tile * 128, -1, np.int64)
        for c in range(64):
            gids = sib_ids_by_combo[c]
            n = len(gids)
            base_n, rem = divmod(n, NC)
            lo = core * base_n + min(core, rem)
            hi = lo + base_n + (1 if core < rem else 0)
            part = gids[lo:hi]
            s0 = sib_tile_off[c] * 128
            assert len(part) <= sib_tpc[c] * 128
            sib_slot[s0:s0 + len(part)] = part
        idx_rows = np.zeros((n_sib_tile, 3, 128), np.int64)
        for t in range(n_sib_tile):
            c = sib_combos[t]
            hc, mc_, sc_ = c // 16, (c // 4) % 4, c % 4
            sel = sib_slot[t * 128:(t + 1) * 128]
            valid = sel >= 0
            sv = np.where(valid, sel, 0)
            idx_rows[t, 0] = np.where(valid, sh_i[sv] - 128 * hc, 0)
            idx_rows[t, 1] = np.where(valid, sm_i[sv] - 128 * mc_, 0)
            idx_rows[t, 2] = np.where(valid, ss_i[sv] - 128 * sc_, 0)
        assert idx_rows.max() < 128 and idx_rows.min() >= 0
        soh = np.zeros((n_sib_tile // 2, 128, 768), _BF)
        svals = idx_rows.reshape(n_sib_tile // 2, 2, 3, 128)
        scols = (np.arange(2)[:, None, None] * 384
                 + np.arange(3)[None, :, None] * 128
                 + np.arange(128)[None, None, :])
        soh[np.arange(n_sib_tile // 2)[:, None, None, None], svals,
            scols[None]] = 1
        # padding tile shares a real combo's one-hot slot: zero it out
        if sib_pad:
            soh[-1, :, 384:768] = 0
        m["sib_oh_in"] = soh
        meta["sib_slots"].append(sib_slot)
        in_maps.append(m)
    return in_maps, meta


LAST_EXEC_NS = None


def kernel(**inputs):
    global LAST_EXEC_NS
    _install_ntff_hook()
    from concourse.bass_utils import run_bass_kernel_spmd

    in_maps, meta = _host_prepare(inputs)
    nc = _get_program(meta["sib_combos"], meta["arc_buckets"])
    import os

    trace = os.environ.get("KERNEL_TRACE", "0") == "1"
    res = run_bass_kernel_spmd(nc, in_maps, list(range(NC)), trace=trace)
    LAST_EXEC_NS = res.exec_time_ns
    _CACHE["res"] = res
    n_sib_tile = len(meta["sib_combos"])
    arc_scores = np.zeros(A, np.float32)
    sib_scores = np.zeros(ASIB, np.float32)
    for core in range(NC):
        sc = np.asarray(res.results[core]["scores_out"])  # [128, n_tile]
        sib_flat = sc[:, :n_sib_tile].T.reshape(-1)
        sib_slot = meta["sib_slots"][core]                # global sib ids
        valid = sib_slot >= 0
        sib_scores[sib_slot[valid]] = sib_flat[valid]

        arc_flat = sc[:, n_sib_tile:].T.reshape(-1)
        arc_slot = meta["arc_slots"][core]                # global arc ids
        valid = arc_slot >= 0
        arc_scores[arc_slot[valid]] = arc_flat[valid]
    return np.concatenate([arc_scores, sib_scores])

